# revision 11
# baseline (speedup 1.0000x reference)
"""Trainium2 Bass kernel for nn_MultiHeadAttention_60078002536549.

Dense transformer block:
    att  = softmax(Q K^T / sqrt(64)) V          (B=2, H=16, N=2048, HD=64)
    x1   = x + att_concat                        (B, N, D=1024)
    out  = x1 + gelu(LN(x1) @ w1 + b1) @ w2 + b2 (FF=4096)

Sharding: tokens are sharded across the 8 cores (core i handles batch i//4,
token rows [512*(i%4), 512*(i%4+1))).  Each core loads the full K/V of its
batch and the full FFN weights; no collectives.

v2: all layout work happens on the HOST (numpy) —
  - K and Q arrive dim-major and head-paired (two 64-dim heads stacked on
    the partition axis), with the 1/sqrt(64) score scale pre-folded into Q
    (exactly representable: 2^-3), so scores S^T = K_pair^T-free @ Q needs
    zero on-chip transposes and exp() needs no scale argument.
  - V arrives token-major per k-chunk with the softmax-denominator ones
    column pre-appended.
  - ln_w/ln_b are folded into w1/b1 (exact, linear), so the LN apply is
    just (x1-mu)*rstd.
  - w1/w2 arrive bf16 in stationary-operand-ready tiling; x arrives
    feature-major; the output is written feature-major and transposed back
    on the host.
On-chip the attention inner loop is the ACT-bound exp stream with score
and AV matmuls (bf16/f16) overlapped; softmax reciprocals run on the DVE
(reciprocal_approx_fast) to keep ACT for exp only; LN stats accumulate on
the PE via a ones-column matmul, deferred one pair to keep the PE dense.
"""

import sys

for _p in ("/opt/trn_rl_repo",):
    if _p not in sys.path:
        sys.path.insert(0, _p)

import ml_dtypes
import numpy as np

import concourse.bass as bass
import concourse.mybir as mybir
import concourse.tile as tile
from concourse.bass import ts
from concourse.bass_utils import run_bass_kernel_spmd

F32 = mybir.dt.float32
F32R = mybir.dt.float32r
BF16 = mybir.dt.bfloat16
F16 = mybir.dt.float16
AF = mybir.ActivationFunctionType

B, H, N, HD, D, FF = 2, 16, 2048, 64, 1024, 4096
NCORES = 8
TOK = (B * N) // NCORES          # 512 tokens per core
SCALE = float(1.0 / np.sqrt(HD))
EPS = 1e-5

KC = N // 128                    # 16 k-token chunks
DC = D // 128                    # 8 feature chunks
FC = FF // 128                   # 32 hidden chunks
NPAIR = H // 2                   # 8 head pairs


def build_program(split_waits=True):
    nc = bass.Bass()

    kp = nc.declare_dram_parameter("kp", [NPAIR, 128, KC, 128], BF16, isOutput=False)
    qp = nc.declare_dram_parameter("qp", [NPAIR, 128, TOK], BF16, isOutput=False)
    vp = nc.declare_dram_parameter("vp", [NPAIR, 2, 128, KC, HD + 1], F16, isOutput=False)
    xt = nc.declare_dram_parameter("xt", [128, DC, TOK], F32, isOutput=False)
    w1p = nc.declare_dram_parameter("w1p", [FC, 128, DC * 128], BF16, isOutput=False)
    b1p = nc.declare_dram_parameter("b1p", [FF], F32, isOutput=False)
    w2p = nc.declare_dram_parameter("w2p", [DC, 128, FC * 128], BF16, isOutput=False)
    b2p = nc.declare_dram_parameter("b2p", [D], F32, isOutput=False)
    y = nc.declare_dram_parameter("y", [DC, 128, TOK], F32, isOutput=True)

    with tile.TileContext(nc) as tc:
        build_tile_kernel(nc, tc, kp, qp, vp, xt, w1p, b1p, w2p, b2p, y)
    if split_waits:
        _split_matmul_waits(nc)
    return nc


def _split_matmul_waits(nc):
    """This walrus build accepts only one sync wait per compute engine
    instruction; move extra waits onto a NoOp inserted right before it on
    the same engine.  DMA/queue instructions are left untouched."""
    for f in nc.m.functions:
        for blk in f.blocks:
            new = []
            for inst in blk.instructions:
                si = inst.sync_info
                if si is not None and len(si.on_wait) > 1:
                    waits = list(si.on_wait)
                    for w in waits[:-1]:
                        new.append(mybir.InstNoOp(
                            name=f"waitsplit_{nc.next_id()}",
                            engine=inst.engine, ins=[], outs=[],
                            sync_info=mybir.SyncInfo(on_wait=[w],
                                                     on_update=[])))
                    inst.sync_info = mybir.SyncInfo(
                        on_wait=waits[-1:], on_update=list(si.on_update))
                new.append(inst)
            blk.instructions[:] = new


def build_tile_kernel(nc, tc, kp, qp, vp, xt, w1p, b1p, w2p, b2p, y):
    from contextlib import ExitStack

    est = ExitStack()
    singles = est.enter_context(tc.tile_pool(name="singles", bufs=1))
    persist = est.enter_context(tc.tile_pool(name="persist", bufs=1))

    # ---- constants / small weights ----
    ones_f32 = singles.tile([128, 1], F32, tag="ones_f32")
    nc.vector.memset(ones_f32, 1.0)
    ones_col = singles.tile([128, 1], F32R, tag="ones_col")
    nc.vector.tensor_copy(out=ones_col, in_=ones_f32)
    eps_t = singles.tile([1, 1], F32, tag="eps")
    nc.vector.memset(eps_t, EPS)

    b1s = singles.tile([128, FC], F32, tag="b1s")
    nc.scalar.dma_start(out=b1s, in_=b1p[:].rearrange("(c p) -> p c", p=128))
    b2s = singles.tile([128, DC], F32, tag="b2s")
    nc.scalar.dma_start(out=b2s, in_=b2p[:].rearrange("(c p) -> p c", p=128))

    # ---- persistent activations ----
    xta = persist.tile([128, DC, TOK], F32, tag="xta")
    x1 = persist.tile([128, DC, TOK], F32R, tag="x1")
    ht = persist.tile([128, DC, TOK], BF16, tag="ht")
    gt = persist.tile([128, FC, TOK], BF16, tag="gt")
    w1a = persist.tile([128, FC, DC * 128], BF16, tag="w1a")

    # =================== attention ===================
    att_est = ExitStack()
    kq_p = att_est.enter_context(tc.tile_pool(name="kq", bufs=2))
    v_p = att_est.enter_context(tc.tile_pool(name="vp", bufs=2))
    e_p = att_est.enter_context(tc.tile_pool(name="ep", bufs=4))
    nrm_p = att_est.enter_context(tc.tile_pool(name="nrm", bufs=2))
    sq_p = att_est.enter_context(tc.tile_pool(name="sqp", bufs=2))
    bcd_p = att_est.enter_context(tc.tile_pool(name="bcd", bufs=2, space="DRAM"))
    s_ps = att_est.enter_context(tc.tile_pool(name="s_ps", bufs=2, space="PSUM"))
    att_ps = att_est.enter_context(tc.tile_pool(name="att_ps", bufs=1, space="PSUM"))

    st_est = ExitStack()
    st_ps = st_est.enter_context(tc.tile_pool(name="st_ps", bufs=1, space="PSUM"))
    stats = st_ps.tile([1, 2, TOK], F32, tag="stats")

    def emit_stats(jprev, sq_tile):
        x1v = x1[:, jprev, :]
        nc.tensor.matmul(stats[:, 0, :], ones_col, x1v,
                         start=(jprev == 0), stop=(jprev == NPAIR - 1))
        nc.tensor.matmul(stats[:, 1, :], ones_col, sq_tile,
                         start=(jprev == 0), stop=(jprev == NPAIR - 1))

    pend_stats = None            # (jprev, sq_tile)
    for j in range(NPAIR):
        # ---- stage K/Q (gpsimd queue), V (sync queue) ----
        kg = kq_p.tile([128, KC, 128], BF16, tag="kg")
        nc.gpsimd.dma_start(out=kg, in_=kp[j])
        qt = kq_p.tile([128, TOK], BF16, tag="qt")
        nc.gpsimd.dma_start(out=qt, in_=qp[j])
        va = v_p.tile([128, KC, HD + 1], F16, tag="va")
        nc.sync.dma_start(out=va, in_=vp[j, 0])
        vb = v_p.tile([128, KC, HD + 1], F16, tag="vb")
        nc.sync.dma_start(out=vb, in_=vp[j, 1])
        # big streaming loads ride the otherwise-idle sync queue, spread
        # across pairs so they never delay the next pair's V tiles
        if j == 0:
            nc.sync.dma_start(out=xta,
                              in_=xt[:].rearrange("p dc t -> p (dc t)")
                              .rearrange("p (dc t) -> p dc t", dc=DC))
        elif 1 <= j <= 4:
            sl = j - 1
            nc.sync.dma_start(
                out=w1a[:, ts(sl, FC // 4), :],
                in_=w1p[ts(sl, FC // 4)].rearrange("f p q -> p f q"))

        att_a = att_ps.tile([HD + 1, TOK], F32, tag="att_a")
        att_b = att_ps.tile([HD + 1, TOK], F32, tag="att_b")

        def exp_av(s, c):
            e = e_p.tile([128, 2, TOK], F16, tag="e")
            nc.scalar.activation(e, s, AF.Exp)
            nc.tensor.matmul(att_a, va[:, c, :], e[:, 0, :],
                             start=(c == 0), stop=(c == KC - 1))
            nc.tensor.matmul(att_b, vb[:, c, :], e[:, 1, :],
                             start=(c == 0), stop=(c == KC - 1))

        pend = None
        for c in range(KC):
            s = s_ps.tile([128, 2, TOK], F32, tag="s")
            nc.tensor.matmul(s[:, 0, :], kg[0:64, c, :], qt[0:64, :],
                             tile_position=(0, 0))
            nc.tensor.matmul(s[:, 1, :], kg[64:128, c, :], qt[64:128, :],
                             tile_position=(64, 0))
            if pend is not None:
                exp_av(*pend)
            pend = (s, c)
            # deferred LN stats for the previous pair, tucked into the
            # second chunk so the PE queue at the pair boundary stays busy
            if c == 2 and pend_stats is not None:
                emit_stats(*pend_stats)
                pend_stats = None
        exp_av(*pend)

        # ---- drain att PSUM to SBUF, reciprocal on DVE, broadcast ----
        cpa = nrm_p.tile([HD + 1, TOK], F32, tag="cpa")
        cpb = nrm_p.tile([HD + 1, TOK], F32, tag="cpb")
        nc.vector.tensor_copy(out=cpa, in_=att_a)
        nc.vector.tensor_copy(out=cpb, in_=att_b)
        # spread the 1024 denominators over all 128 partitions via a DRAM
        # round-trip so the DVE's iterative divide runs 8 elems/lane
        # (~0.25us) instead of 512 on one lane (~3.3us each)
        bcd = bcd_p.tile([2, TOK], F32, tag="bcd")
        nc.gpsimd.dma_start(out=bcd[0:1, :], in_=cpa[HD:HD + 1, :])
        nc.gpsimd.dma_start(out=bcd[1:2, :], in_=cpb[HD:HD + 1, :])
        den128 = nrm_p.tile([128, 8], F32, tag="den128")
        nc.gpsimd.dma_start(
            out=den128, in_=bcd[:].rearrange("a (b f) -> (a b) f", f=8))
        rec128 = nrm_p.tile([128, 8], F32, tag="rec128")
        nc.vector.reciprocal(out=rec128, in_=den128)
        bcd2 = bcd_p.tile([2, TOK], F32, tag="bcd2")
        nc.gpsimd.dma_start(
            out=bcd2[:].rearrange("a (b f) -> (a b) f", f=8), in_=rec128)
        bca = nrm_p.tile([HD, TOK], F32, tag="bca")
        bcb = nrm_p.tile([HD, TOK], F32, tag="bcb")
        nc.gpsimd.dma_start(out=bca, in_=bcd2[0:1, :].to_broadcast((HD, TOK)))
        nc.gpsimd.dma_start(out=bcb, in_=bcd2[1:2, :].to_broadcast((HD, TOK)))

        # ---- normalize + residual into x1 (feature block j) ----
        nc.vector.tensor_mul(x1[0:HD, j, :], cpa[0:HD, :], bca)
        nc.vector.tensor_mul(x1[HD:128, j, :], cpb[0:HD, :], bcb)
        nc.vector.tensor_add(x1[0:HD, j, :], x1[0:HD, j, :], xta[0:HD, j, :])
        nc.vector.tensor_add(x1[HD:128, j, :], x1[HD:128, j, :],
                             xta[HD:128, j, :])

        # squares for the LN variance (DVE, off the ACT critical path)
        sq = sq_p.tile([128, TOK], F32R, tag="sq")
        nc.vector.tensor_mul(sq, x1[:, j, :], x1[:, j, :])
        pend_stats = (j, sq)

    emit_stats(*pend_stats)

    # ---- layer-norm scalars ----
    mu = persist.tile([1, TOK], F32, tag="mu")
    msq = persist.tile([1, TOK], F32, tag="msq")
    var = persist.tile([1, TOK], F32, tag="var")
    rstd = persist.tile([1, TOK], F32, tag="rstd")
    nc.vector.tensor_scalar_mul(mu, stats[:, 0, :], 1.0 / D)
    nc.vector.tensor_scalar_mul(msq, stats[:, 1, :], 1.0 / D)
    st_est.close()
    nc.vector.tensor_mul(var, mu, mu)
    nc.vector.tensor_sub(var, msq, var)
    # rstd = exp(-0.5 * ln(var + eps)) -- stays within the ln/exp table set
    nc.scalar.activation(var, var, AF.Ln, bias=eps_t)
    nc.scalar.activation(rstd, var, AF.Exp, scale=-0.5)

    mu_b = persist.tile([128, TOK], F32, tag="mu_b")
    rstd_b = persist.tile([128, TOK], F32, tag="rstd_b")
    lnd = bcd_p.tile([2, TOK], F32, tag="lnd")
    nc.gpsimd.dma_start(out=lnd[0:1, :], in_=mu)
    nc.gpsimd.dma_start(out=lnd[1:2, :], in_=rstd)
    nc.gpsimd.dma_start(out=mu_b, in_=lnd[0:1, :].to_broadcast((128, TOK)))
    nc.gpsimd.dma_start(out=rstd_b, in_=lnd[1:2, :].to_broadcast((128, TOK)))

    att_est.close()

    # =================== FFN ===================
    ffn_est = ExitStack()
    mm_ps = ffn_est.enter_context(tc.tile_pool(name="mm_ps", bufs=6, space="PSUM"))
    ln_p = ffn_est.enter_context(tc.tile_pool(name="ln", bufs=2))
    w2_p = ffn_est.enter_context(tc.tile_pool(name="w2p", bufs=2))
    o_p = ffn_est.enter_context(tc.tile_pool(name="op", bufs=2))

    # LN apply: ht = (x1 - mu) * rstd   (ln_w/ln_b folded into w1/b1)
    for jj in range(DC):
        t = ln_p.tile([128, TOK], F32, tag="lnt")
        nc.vector.tensor_sub(t, x1[:, jj, :], mu_b)
        nc.vector.tensor_mul(ht[:, jj, :], t, rstd_b)

    # FFN1: gt[fc] = gelu(w1[:,fc]^T h + b1[fc])
    for f in range(FC):
        ps = mm_ps.tile([128, TOK], F32, tag="mm")
        for dc in range(DC):
            nc.tensor.matmul(ps, w1a[:, f, ts(dc, 128)], ht[:, dc, :],
                             start=(dc == 0), stop=(dc == DC - 1))
        nc.scalar.activation(gt[:, f, :], ps, AF.Gelu, bias=b1s[:, f:f + 1])

    # FFN2: y[dd] = w2[:,dd]^T g + b2[dd] + x1[dd]
    for dd in range(DC):
        w2t = w2_p.tile([128, FC * 128], BF16, tag="w2t")
        nc.sync.dma_start(out=w2t, in_=w2p[dd])
        ps = mm_ps.tile([128, TOK], F32, tag="mm")
        for fc in range(FC):
            nc.tensor.matmul(ps, w2t[:, ts(fc, 128)], gt[:, fc, :],
                             start=(fc == 0), stop=(fc == FC - 1))
        o = o_p.tile([128, TOK], F32, tag="o")
        nc.vector.tensor_scalar_add(o, ps, b2s[:, dd:dd + 1])
        nc.vector.tensor_add(o, o, x1[:, dd, :])
        nc.sync.dma_start(out=y[dd], in_=o)

    ffn_est.close()
    est.close()


_PROGRAMS = {}


def get_program(split_waits=True):
    if split_waits not in _PROGRAMS:
        _PROGRAMS[split_waits] = build_program(split_waits)
    return _PROGRAMS[split_waits]


def make_in_maps(x, image_q, image_k, image_v, ln_w, ln_b, w1, b1, w2, b2):
    f32 = np.float32
    bf16 = ml_dtypes.bfloat16
    x = np.asarray(x, f32)
    image_q = np.asarray(image_q, f32)
    image_k = np.asarray(image_k, f32)
    image_v = np.asarray(image_v, f32)
    w1 = np.asarray(w1, f32)
    b1 = np.asarray(b1, f32)
    w2 = np.asarray(w2, f32)
    b2 = np.asarray(b2, f32)
    ln_w = np.asarray(ln_w, f32)
    ln_b = np.asarray(ln_b, f32)

    # fold LN affine into w1/b1 (exact)
    w1f = ln_w[:, None] * w1
    b1f = b1 + ln_b @ w1
    w1p = np.ascontiguousarray(
        w1f.reshape(DC, 128, FC, 128).transpose(2, 1, 0, 3)
        .reshape(FC, 128, DC * 128).astype(bf16))
    w2p = np.ascontiguousarray(
        w2.reshape(FC, 128, DC, 128).transpose(2, 1, 0, 3)
        .reshape(DC, 128, FC * 128).astype(bf16))

    shared = {"w1p": w1p, "b1p": b1f, "w2p": w2p, "b2p": b2}

    in_maps = []
    for core in range(NCORES):
        b, r = divmod(core, NCORES // B)
        rows = slice(TOK * r, TOK * (r + 1))
        # K dim-major, head-paired: [NPAIR, 128, KC, 128]
        kpa = np.ascontiguousarray(
            image_k[b].reshape(NPAIR, 2, KC, 128, HD)
            .transpose(0, 1, 4, 2, 3).reshape(NPAIR, 128, KC, 128)
            .astype(bf16))
        # Q dim-major, head-paired, scale folded: [NPAIR, 128, TOK]
        qpa = np.ascontiguousarray(
            (image_q[b, :, rows] * SCALE).reshape(NPAIR, 2, TOK, HD)
            .transpose(0, 1, 3, 2).reshape(NPAIR, 128, TOK)
            .astype(bf16))
        # V token-major per chunk with ones column: [NPAIR, 2, 128, KC, 65]
        vv = image_v[b].reshape(NPAIR, 2, KC, 128, HD).transpose(0, 1, 3, 2, 4)
        vpa = np.empty((NPAIR, 2, 128, KC, HD + 1), np.float16)
        vpa[..., :HD] = vv
        vpa[..., HD] = 1.0
        # x feature-major: [128, DC, TOK]
        xta = np.ascontiguousarray(
            x[b, rows].T.reshape(DC, 128, TOK).transpose(1, 0, 2))
        in_maps.append({
            "kp": kpa, "qp": qpa, "vp": vpa, "xt": xta, **shared,
        })
    return in_maps


def run_cores(in_maps, trace=False, **kw):
    nc = get_program()
    return run_bass_kernel_spmd(nc, in_maps, core_ids=list(range(NCORES)),
                                trace=trace, **kw)


def kernel(x, image_q, image_k, image_v, ln_w, ln_b, w1, b1, w2, b2):
    in_maps = make_in_maps(x, image_q, image_k, image_v, ln_w, ln_b,
                           w1, b1, w2, b2)
    res = run_cores(in_maps)
    out = np.empty((B, N, D), dtype=np.float32)
    for core in range(NCORES):
        b, r = divmod(core, NCORES // B)
        out[b, TOK * r:TOK * (r + 1)] = \
            np.asarray(res.results[core]["y"]).reshape(D, TOK).T
    return out


# revision 17
# speedup vs baseline: 1.1663x; 1.1663x over previous
"""Trainium2 Bass kernel for nn_MultiHeadAttention_60078002536549.

Dense transformer block:
    att  = softmax(Q K^T / sqrt(64)) V          (B=2, H=16, N=2048, HD=64)
    x1   = x + att_concat                        (B, N, D=1024)
    out  = x1 + gelu(LN(x1) @ w1 + b1) @ w2 + b2 (FF=4096)

Sharding: tokens are sharded across the 8 cores (core i handles batch i//4,
token rows [512*(i%4), 512*(i%4+1))).  Each core loads the full K/V of its
batch and the full FFN weights; no collectives.

v2: all layout work happens on the HOST (numpy) —
  - K and Q arrive dim-major and head-paired (two 64-dim heads stacked on
    the partition axis), with the 1/sqrt(64) score scale pre-folded into Q
    (exactly representable: 2^-3), so scores S^T = K_pair^T-free @ Q needs
    zero on-chip transposes and exp() needs no scale argument.
  - V arrives token-major per k-chunk with the softmax-denominator ones
    column pre-appended.
  - ln_w/ln_b are folded into w1/b1 (exact, linear), so the LN apply is
    just (x1-mu)*rstd.
  - w1/w2 arrive bf16 in stationary-operand-ready tiling; x arrives
    feature-major; the output is written feature-major and transposed back
    on the host.
On-chip the attention inner loop is the ACT-bound exp stream with score
and AV matmuls (bf16/f16) overlapped; softmax reciprocals run on the DVE
(reciprocal_approx_fast) to keep ACT for exp only; LN stats accumulate on
the PE via a ones-column matmul, deferred one pair to keep the PE dense.
"""

import sys

for _p in ("/opt/trn_rl_repo",):
    if _p not in sys.path:
        sys.path.insert(0, _p)

import ml_dtypes
import numpy as np

import concourse.bass as bass
import concourse.mybir as mybir
import concourse.tile as tile
from concourse.bass import ts
from concourse.bass_utils import run_bass_kernel_spmd

F32 = mybir.dt.float32
F32R = mybir.dt.float32r
BF16 = mybir.dt.bfloat16
F16 = mybir.dt.float16
AF = mybir.ActivationFunctionType

B, H, N, HD, D, FF = 2, 16, 2048, 64, 1024, 4096
NCORES = 8
TOK = (B * N) // NCORES          # 512 tokens per core
SCALE = float(1.0 / np.sqrt(HD))
EPS = 1e-5

KC = N // 128                    # 16 k-token chunks
DC = D // 128                    # 8 feature chunks
FC = FF // 128                   # 32 hidden chunks
NPAIR = H // 2                   # 8 head pairs


def build_program(split_waits=True):
    nc = bass.Bass()

    kp = nc.declare_dram_parameter("kp", [NPAIR, 128, KC, 128], BF16, isOutput=False)
    qp = nc.declare_dram_parameter("qp", [NPAIR, 128, TOK], BF16, isOutput=False)
    vp = nc.declare_dram_parameter("vp", [NPAIR, 2, 128, KC, HD + 1], F16, isOutput=False)
    xt = nc.declare_dram_parameter("xt", [128, DC, TOK], F32, isOutput=False)
    w1p = nc.declare_dram_parameter("w1p", [FC, 128, DC * 128], BF16, isOutput=False)
    b1p = nc.declare_dram_parameter("b1p", [128, FC], F32, isOutput=False)
    w2p = nc.declare_dram_parameter("w2p", [DC, 128, FC * 128], BF16, isOutput=False)
    b2p = nc.declare_dram_parameter("b2p", [128, DC], F32, isOutput=False)
    y = nc.declare_dram_parameter("y", [DC, 128, TOK], F32, isOutput=True)

    with tile.TileContext(nc) as tc:
        build_tile_kernel(nc, tc, kp, qp, vp, xt, w1p, b1p, w2p, b2p, y)
    if split_waits:
        _split_matmul_waits(nc)
    return nc


def _split_matmul_waits(nc):
    """This walrus build accepts only one sync wait per compute engine
    instruction; move extra waits onto a NoOp inserted right before it on
    the same engine.  DMA/queue instructions are left untouched."""
    for f in nc.m.functions:
        for blk in f.blocks:
            new = []
            for inst in blk.instructions:
                si = inst.sync_info
                if si is not None and len(si.on_wait) > 1:
                    waits = list(si.on_wait)
                    for w in waits[:-1]:
                        new.append(mybir.InstNoOp(
                            name=f"waitsplit_{nc.next_id()}",
                            engine=inst.engine, ins=[], outs=[],
                            sync_info=mybir.SyncInfo(on_wait=[w],
                                                     on_update=[])))
                    inst.sync_info = mybir.SyncInfo(
                        on_wait=waits[-1:], on_update=list(si.on_update))
                new.append(inst)
            blk.instructions[:] = new


def build_tile_kernel(nc, tc, kp, qp, vp, xt, w1p, b1p, w2p, b2p, y):
    from contextlib import ExitStack

    est = ExitStack()
    singles = est.enter_context(tc.tile_pool(name="singles", bufs=1))
    persist = est.enter_context(tc.tile_pool(name="persist", bufs=1))

    # ---- constants / small weights ----
    ones_f32 = singles.tile([128, 1], F32, tag="ones_f32")
    nc.vector.memset(ones_f32, 1.0)
    ones_col = singles.tile([128, 1], F32R, tag="ones_col")
    nc.vector.tensor_copy(out=ones_col, in_=ones_f32)
    eps_t = singles.tile([1, 1], F32, tag="eps")
    nc.vector.memset(eps_t, EPS)

    b1s = singles.tile([128, FC], F32, tag="b1s")
    nc.scalar.dma_start(out=b1s, in_=b1p[:])
    b2s = singles.tile([128, DC], F32, tag="b2s")
    nc.scalar.dma_start(out=b2s, in_=b2p[:])

    # ---- persistent activations ----
    xta = persist.tile([128, DC, TOK], F32, tag="xta")
    x1 = persist.tile([128, DC, TOK], F32R, tag="x1")
    ht = persist.tile([128, DC, TOK], BF16, tag="ht")
    gt = persist.tile([128, FC, TOK], BF16, tag="gt")
    w1a = persist.tile([128, FC, DC * 128], BF16, tag="w1a")

    # =================== attention ===================
    att_est = ExitStack()
    kq_p = att_est.enter_context(tc.tile_pool(name="kq", bufs=2))
    v_p = att_est.enter_context(tc.tile_pool(name="vp", bufs=2))
    e_p = att_est.enter_context(tc.tile_pool(name="ep", bufs=4))
    nrm_p = att_est.enter_context(tc.tile_pool(name="nrm", bufs=2))
    sq_p = att_est.enter_context(tc.tile_pool(name="sqp", bufs=2))
    bcd_p = att_est.enter_context(tc.tile_pool(name="bcd", bufs=2, space="DRAM"))
    ps_est = ExitStack()
    s_ps = ps_est.enter_context(tc.tile_pool(name="s_ps", bufs=2, space="PSUM"))
    att_ps = ps_est.enter_context(tc.tile_pool(name="att_ps", bufs=1, space="PSUM"))

    st_est = ExitStack()
    st_ps = st_est.enter_context(tc.tile_pool(name="st_ps", bufs=1, space="PSUM"))
    stats = st_ps.tile([1, 2, TOK], F32, tag="stats")

    def emit_stats(jprev, sq_tile):
        x1v = x1[:, jprev, :]
        nc.tensor.matmul(stats[:, 0, :], ones_col, x1v,
                         start=(jprev == 0), stop=(jprev == NPAIR - 1))
        nc.tensor.matmul(stats[:, 1, :], ones_col, sq_tile,
                         start=(jprev == 0), stop=(jprev == NPAIR - 1))

    pend_stats = None            # (jprev, sq_tile)
    for j in range(NPAIR):
        # ---- stage K/Q (gpsimd queue), V (sync queue) ----
        qt = kq_p.tile([128, TOK], BF16, tag="qt")
        nc.gpsimd.dma_start(out=qt, in_=qp[j])
        kg = kq_p.tile([128, KC, 128], BF16, tag="kg")
        nc.gpsimd.dma_start(out=kg[:, 0:KC // 2, :], in_=kp[j, :, 0:KC // 2])
        nc.gpsimd.dma_start(out=kg[:, KC // 2:, :], in_=kp[j, :, KC // 2:])
        va = v_p.tile([128, KC, HD + 1], F16, tag="va")
        nc.sync.dma_start(out=va, in_=vp[j, 0])
        vb = v_p.tile([128, KC, HD + 1], F16, tag="vb")
        nc.sync.dma_start(out=vb, in_=vp[j, 1])
        # big streaming loads ride the otherwise-idle sync queue, spread
        # across pairs so they never delay the next pair's V tiles
        if j == 0:
            nc.sync.dma_start(out=xta,
                              in_=xt[:].rearrange("p dc t -> p (dc t)")
                              .rearrange("p (dc t) -> p dc t", dc=DC))
        elif 1 <= j <= 4:
            sl = j - 1
            nc.sync.dma_start(
                out=w1a[:, ts(sl, FC // 4), :],
                in_=w1p[ts(sl, FC // 4)].rearrange("f p q -> p f q"))

        att_a = att_ps.tile([HD + 1, TOK], F32, tag="att_a")
        att_b = att_ps.tile([HD + 1, TOK], F32, tag="att_b")

        def exp_av(s, c):
            e = e_p.tile([128, 2, TOK], F16, tag="e")
            nc.scalar.activation(e, s, AF.Exp)
            nc.tensor.matmul(att_a, va[:, c, :], e[:, 0, :],
                             start=(c == 0), stop=(c == KC - 1))
            nc.tensor.matmul(att_b, vb[:, c, :], e[:, 1, :],
                             start=(c == 0), stop=(c == KC - 1))

        pend = None
        for c in range(KC):
            s = s_ps.tile([128, 2, TOK], F32, tag="s")
            nc.tensor.matmul(s[:, 0, :], kg[0:64, c, :], qt[0:64, :],
                             tile_position=(0, 0))
            nc.tensor.matmul(s[:, 1, :], kg[64:128, c, :], qt[64:128, :],
                             tile_position=(64, 0))
            if pend is not None:
                exp_av(*pend)
            pend = (s, c)
            # deferred LN stats for the previous pair, tucked into the
            # second chunk so the PE queue at the pair boundary stays busy
            if c == 2 and pend_stats is not None:
                emit_stats(*pend_stats)
                pend_stats = None
        exp_av(*pend)

        # ---- drain att PSUM to SBUF, reciprocal on DVE, broadcast ----
        cpa = nrm_p.tile([HD + 1, TOK], F32, tag="cpa")
        cpb = nrm_p.tile([HD + 1, TOK], F32, tag="cpb")
        nc.vector.tensor_copy(out=cpa, in_=att_a)
        nc.vector.tensor_copy(out=cpb, in_=att_b)
        # spread the 1024 denominators over all 128 partitions via a DRAM
        # round-trip so the DVE's iterative divide runs 8 elems/lane
        # (~0.25us) instead of 512 on one lane (~3.3us each).  The DMA-hop
        # latencies hide under the next pair's compute — except on the last
        # pair, where the short direct-DVE chain wins.
        bcd2 = bcd_p.tile([2, TOK], F32, tag="bcd2")
        if j < NPAIR - 1:
            bcd = bcd_p.tile([2, TOK], F32, tag="bcd")
            nc.gpsimd.dma_start(out=bcd[0:1, :], in_=cpa[HD:HD + 1, :])
            nc.gpsimd.dma_start(out=bcd[1:2, :], in_=cpb[HD:HD + 1, :])
            den128 = nrm_p.tile([128, 8], F32, tag="den128")
            nc.gpsimd.dma_start(
                out=den128, in_=bcd[:].rearrange("a (b f) -> (a b) f", f=8))
            rec128 = nrm_p.tile([128, 8], F32, tag="rec128")
            nc.vector.reciprocal(out=rec128, in_=den128)
            nc.gpsimd.dma_start(
                out=bcd2[:].rearrange("a (b f) -> (a b) f", f=8), in_=rec128)
        else:
            rab = nrm_p.tile([1, 2, TOK], F32, tag="rab")
            nc.vector.reciprocal(out=rab[:, 0, :], in_=cpa[HD:HD + 1, :])
            nc.vector.reciprocal(out=rab[:, 1, :], in_=cpb[HD:HD + 1, :])
            nc.gpsimd.dma_start(out=bcd2, in_=rab)
        bca = nrm_p.tile([HD, TOK], F32, tag="bca")
        bcb = nrm_p.tile([HD, TOK], F32, tag="bcb")
        nc.gpsimd.dma_start(out=bca, in_=bcd2[0:1, :].to_broadcast((HD, TOK)))
        nc.gpsimd.dma_start(out=bcb, in_=bcd2[1:2, :].to_broadcast((HD, TOK)))

        # ---- normalize + residual into x1 (feature block j) ----
        nc.vector.tensor_mul(x1[0:HD, j, :], cpa[0:HD, :], bca)
        nc.vector.tensor_mul(x1[HD:128, j, :], cpb[0:HD, :], bcb)
        nc.vector.tensor_add(x1[0:HD, j, :], x1[0:HD, j, :], xta[0:HD, j, :])
        nc.vector.tensor_add(x1[HD:128, j, :], x1[HD:128, j, :],
                             xta[HD:128, j, :])

        # squares for the LN variance (DVE, off the ACT critical path)
        sq = sq_p.tile([128, TOK], F32R, tag="sq")
        nc.vector.tensor_mul(sq, x1[:, j, :], x1[:, j, :])
        pend_stats = (j, sq)

    emit_stats(*pend_stats)

    # ---- layer-norm scalars ----
    mu = persist.tile([1, TOK], F32, tag="mu")
    msq = persist.tile([1, TOK], F32, tag="msq")
    var = persist.tile([1, TOK], F32, tag="var")
    rstd = persist.tile([1, TOK], F32, tag="rstd")
    nc.vector.tensor_scalar_mul(mu, stats[:, 0, :], 1.0 / D)
    nc.vector.tensor_scalar_mul(msq, stats[:, 1, :], 1.0 / D)
    st_est.close()
    nc.vector.tensor_mul(var, mu, mu)
    nc.vector.tensor_sub(var, msq, var)
    # rstd = exp(-0.5 * ln(var + eps)) -- stays within the ln/exp table set
    nc.scalar.activation(var, var, AF.Ln, bias=eps_t)
    nc.scalar.activation(rstd, var, AF.Exp, scale=-0.5)

    mu_b = persist.tile([128, TOK], F32, tag="mu_b")
    rstd_b = persist.tile([128, TOK], F32, tag="rstd_b")
    lnd = bcd_p.tile([2, TOK], F32, tag="lnd")
    nc.gpsimd.dma_start(out=lnd[0:1, :], in_=mu)
    nc.gpsimd.dma_start(out=lnd[1:2, :], in_=rstd)
    nc.gpsimd.dma_start(out=mu_b, in_=lnd[0:1, :].to_broadcast((128, TOK)))
    nc.gpsimd.dma_start(out=rstd_b, in_=lnd[1:2, :].to_broadcast((128, TOK)))

    ps_est.close()
    att_est.close()

    # =================== FFN ===================
    ffn_est = ExitStack()
    mm_ps = ffn_est.enter_context(tc.tile_pool(name="mm_ps", bufs=6, space="PSUM"))
    ln_p = ffn_est.enter_context(tc.tile_pool(name="ln", bufs=2))
    w2_p = ffn_est.enter_context(tc.tile_pool(name="w2p", bufs=2))
    o_p = ffn_est.enter_context(tc.tile_pool(name="op", bufs=2))

    # LN apply: ht = (x1 - mu) * rstd   (ln_w/ln_b folded into w1/b1)
    for jj in range(DC):
        t = ln_p.tile([128, TOK], F32, tag="lnt")
        nc.vector.tensor_sub(t, x1[:, jj, :], mu_b)
        nc.vector.tensor_mul(ht[:, jj, :], t, rstd_b)

    # FFN1: gt[fc] = gelu(w1[:,fc]^T h + b1[fc])
    for f in range(FC):
        ps = mm_ps.tile([128, TOK], F32, tag="mm")
        for dc in range(DC):
            nc.tensor.matmul(ps, w1a[:, f, ts(dc, 128)], ht[:, dc, :],
                             start=(dc == 0), stop=(dc == DC - 1))
        nc.scalar.activation(gt[:, f, :], ps, AF.Gelu, bias=b1s[:, f:f + 1])

    # FFN2: y[dd] = w2[:,dd]^T g + b2[dd] + x1[dd]
    for dd in range(DC):
        w2t = w2_p.tile([128, FC * 128], BF16, tag="w2t")
        nc.sync.dma_start(out=w2t, in_=w2p[dd])
        ps = mm_ps.tile([128, TOK], F32, tag="mm")
        for fc in range(FC):
            nc.tensor.matmul(ps, w2t[:, ts(fc, 128)], gt[:, fc, :],
                             start=(fc == 0), stop=(fc == FC - 1))
        o = o_p.tile([128, TOK], F32, tag="o")
        nc.vector.tensor_scalar_add(o, ps, b2s[:, dd:dd + 1])
        nc.vector.tensor_add(o, o, x1[:, dd, :])
        nc.sync.dma_start(out=y[dd], in_=o)

    ffn_est.close()
    est.close()


_PROGRAMS = {}


def get_program(split_waits=True):
    if split_waits not in _PROGRAMS:
        _PROGRAMS[split_waits] = build_program(split_waits)
    return _PROGRAMS[split_waits]


def make_in_maps(x, image_q, image_k, image_v, ln_w, ln_b, w1, b1, w2, b2):
    f32 = np.float32
    bf16 = ml_dtypes.bfloat16
    x = np.asarray(x, f32)
    image_q = np.asarray(image_q, f32)
    image_k = np.asarray(image_k, f32)
    image_v = np.asarray(image_v, f32)
    w1 = np.asarray(w1, f32)
    b1 = np.asarray(b1, f32)
    w2 = np.asarray(w2, f32)
    b2 = np.asarray(b2, f32)
    ln_w = np.asarray(ln_w, f32)
    ln_b = np.asarray(ln_b, f32)

    # fold LN affine into w1/b1 (exact)
    w1f = ln_w[:, None] * w1
    b1f = b1 + ln_b @ w1
    w1p = np.ascontiguousarray(
        w1f.reshape(DC, 128, FC, 128).transpose(2, 1, 0, 3)
        .reshape(FC, 128, DC * 128).astype(bf16))
    w2p = np.ascontiguousarray(
        w2.reshape(FC, 128, DC, 128).transpose(2, 1, 0, 3)
        .reshape(DC, 128, FC * 128).astype(bf16))

    shared = {"w1p": w1p,
          "b1p": np.ascontiguousarray(b1f.reshape(FC, 128).T),
          "w2p": w2p,
          "b2p": np.ascontiguousarray(b2.reshape(DC, 128).T)}

    in_maps = []
    for core in range(NCORES):
        b, r = divmod(core, NCORES // B)
        rows = slice(TOK * r, TOK * (r + 1))
        # K dim-major, head-paired: [NPAIR, 128, KC, 128]
        kpa = np.ascontiguousarray(
            image_k[b].reshape(NPAIR, 2, KC, 128, HD)
            .transpose(0, 1, 4, 2, 3).reshape(NPAIR, 128, KC, 128)
            .astype(bf16))
        # Q dim-major, head-paired, scale folded: [NPAIR, 128, TOK]
        qpa = np.ascontiguousarray(
            (image_q[b, :, rows] * SCALE).reshape(NPAIR, 2, TOK, HD)
            .transpose(0, 1, 3, 2).reshape(NPAIR, 128, TOK)
            .astype(bf16))
        # V token-major per chunk with ones column: [NPAIR, 2, 128, KC, 65]
        vv = image_v[b].reshape(NPAIR, 2, KC, 128, HD).transpose(0, 1, 3, 2, 4)
        vpa = np.empty((NPAIR, 2, 128, KC, HD + 1), np.float16)
        vpa[..., :HD] = vv
        vpa[..., HD] = 1.0
        # x feature-major: [128, DC, TOK]
        xta = np.ascontiguousarray(
            x[b, rows].T.reshape(DC, 128, TOK).transpose(1, 0, 2))
        in_maps.append({
            "kp": kpa, "qp": qpa, "vp": vpa, "xt": xta, **shared,
        })
    return in_maps


def run_cores(in_maps, trace=False, **kw):
    nc = get_program()
    return run_bass_kernel_spmd(nc, in_maps, core_ids=list(range(NCORES)),
                                trace=trace, **kw)


def kernel(x, image_q, image_k, image_v, ln_w, ln_b, w1, b1, w2, b2):
    in_maps = make_in_maps(x, image_q, image_k, image_v, ln_w, ln_b,
                           w1, b1, w2, b2)
    res = run_cores(in_maps)
    out = np.empty((B, N, D), dtype=np.float32)
    for core in range(NCORES):
        b, r = divmod(core, NCORES // B)
        out[b, TOK * r:TOK * (r + 1)] = \
            np.asarray(res.results[core]["y"]).reshape(D, TOK).T
    return out


# revision 22
# speedup vs baseline: 1.2252x; 1.0505x over previous
"""Trainium2 Bass kernel for nn_MultiHeadAttention_60078002536549.

Dense transformer block:
    att  = softmax(Q K^T / sqrt(64)) V          (B=2, H=16, N=2048, HD=64)
    x1   = x + att_concat                        (B, N, D=1024)
    out  = x1 + gelu(LN(x1) @ w1 + b1) @ w2 + b2 (FF=4096)

Sharding: tokens are sharded across the 8 cores (core i handles batch i//4,
token rows [512*(i%4), 512*(i%4+1))).  Each core loads the full K/V of its
batch and the full FFN weights; no collectives.

v2: all layout work happens on the HOST (numpy) —
  - K and Q arrive dim-major and head-paired (two 64-dim heads stacked on
    the partition axis), with the 1/sqrt(64) score scale pre-folded into Q
    (exactly representable: 2^-3), so scores S^T = K_pair^T-free @ Q needs
    zero on-chip transposes and exp() needs no scale argument.
  - V arrives token-major per k-chunk with the softmax-denominator ones
    column pre-appended.
  - ln_w/ln_b are folded into w1/b1 (exact, linear), so the LN apply is
    just (x1-mu)*rstd.
  - w1/w2 arrive bf16 in stationary-operand-ready tiling; x arrives
    feature-major; the output is written feature-major and transposed back
    on the host.
On-chip the attention inner loop is the ACT-bound exp stream with score
and AV matmuls (bf16/f16) overlapped; softmax reciprocals run on the DVE
(reciprocal_approx_fast) to keep ACT for exp only; LN stats accumulate on
the PE via a ones-column matmul, deferred one pair to keep the PE dense.
"""

import sys

for _p in ("/opt/trn_rl_repo",):
    if _p not in sys.path:
        sys.path.insert(0, _p)

import ml_dtypes
import numpy as np

import concourse.bass as bass
import concourse.mybir as mybir
import concourse.tile as tile
from concourse.bass import ts
from concourse.bass_utils import run_bass_kernel_spmd

F32 = mybir.dt.float32
F32R = mybir.dt.float32r
BF16 = mybir.dt.bfloat16
F16 = mybir.dt.float16
AF = mybir.ActivationFunctionType

B, H, N, HD, D, FF = 2, 16, 2048, 64, 1024, 4096
NCORES = 8
TOK = (B * N) // NCORES          # 512 tokens per core
SCALE = float(1.0 / np.sqrt(HD))
EPS = 1e-5

KC = N // 128                    # 16 k-token chunks
DC = D // 128                    # 8 feature chunks
FC = FF // 128                   # 32 hidden chunks
NPAIR = H // 2                   # 8 head pairs


def build_program(split_waits=True):
    nc = bass.Bass()

    kp = nc.declare_dram_parameter("kp", [NPAIR, 128, KC, 128], BF16, isOutput=False)
    qp = nc.declare_dram_parameter("qp", [NPAIR, 128, TOK], BF16, isOutput=False)
    vp = nc.declare_dram_parameter("vp", [NPAIR, 2, 128, KC, HD + 1], F16, isOutput=False)
    xt = nc.declare_dram_parameter("xt", [128, DC, TOK], F32, isOutput=False)
    w1p = nc.declare_dram_parameter("w1p", [FC, 128, DC * 128], BF16, isOutput=False)
    b1p = nc.declare_dram_parameter("b1p", [128, FC], F32, isOutput=False)
    w2p = nc.declare_dram_parameter("w2p", [DC, 128, FC * 128], BF16, isOutput=False)
    b2p = nc.declare_dram_parameter("b2p", [128, DC], F32, isOutput=False)
    y = nc.declare_dram_parameter("y", [DC, 128, TOK], F32, isOutput=True)

    with tile.TileContext(nc) as tc:
        build_tile_kernel(nc, tc, kp, qp, vp, xt, w1p, b1p, w2p, b2p, y)
    if split_waits:
        _split_matmul_waits(nc)
    return nc


def _split_matmul_waits(nc):
    """This walrus build accepts only one sync wait per compute engine
    instruction; move extra waits onto a NoOp inserted right before it on
    the same engine.  DMA/queue instructions are left untouched."""
    for f in nc.m.functions:
        for blk in f.blocks:
            new = []
            for inst in blk.instructions:
                si = inst.sync_info
                if si is not None and len(si.on_wait) > 1:
                    waits = list(si.on_wait)
                    for w in waits[:-1]:
                        new.append(mybir.InstNoOp(
                            name=f"waitsplit_{nc.next_id()}",
                            engine=inst.engine, ins=[], outs=[],
                            sync_info=mybir.SyncInfo(on_wait=[w],
                                                     on_update=[])))
                    inst.sync_info = mybir.SyncInfo(
                        on_wait=waits[-1:], on_update=list(si.on_update))
                new.append(inst)
            blk.instructions[:] = new


def build_tile_kernel(nc, tc, kp, qp, vp, xt, w1p, b1p, w2p, b2p, y):
    from contextlib import ExitStack

    est = ExitStack()
    singles = est.enter_context(tc.tile_pool(name="singles", bufs=1))
    persist = est.enter_context(tc.tile_pool(name="persist", bufs=1))

    # ---- constants / small weights ----
    ones_f32 = singles.tile([128, 1], F32, tag="ones_f32")
    nc.vector.memset(ones_f32, 1.0)
    ones_col = singles.tile([128, 1], F32R, tag="ones_col")
    nc.vector.tensor_copy(out=ones_col, in_=ones_f32)
    ones_rf = singles.tile([1, 128], F32, tag="ones_rf")
    nc.vector.memset(ones_rf, 1.0)
    ones_row = singles.tile([1, 128], F32R, tag="ones_row")
    nc.vector.tensor_copy(out=ones_row, in_=ones_rf)
    eps_t = singles.tile([1, 1], F32, tag="eps")
    nc.vector.memset(eps_t, EPS)

    b1s = singles.tile([128, FC], F32, tag="b1s")
    nc.scalar.dma_start(out=b1s, in_=b1p[:])
    b2s = singles.tile([128, DC], F32, tag="b2s")
    nc.scalar.dma_start(out=b2s, in_=b2p[:])

    # ---- persistent activations ----
    xta = persist.tile([128, DC, TOK], F32, tag="xta")
    x1 = persist.tile([128, DC, TOK], F32R, tag="x1")
    ht = persist.tile([128, DC, TOK], BF16, tag="ht")
    gt = persist.tile([128, FC, TOK], BF16, tag="gt")
    w1a = persist.tile([128, FC, DC * 128], BF16, tag="w1a")

    # =================== attention ===================
    att_est = ExitStack()
    kq_p = att_est.enter_context(tc.tile_pool(name="kq", bufs=2))
    v_p = att_est.enter_context(tc.tile_pool(name="vp", bufs=2))
    e_p = att_est.enter_context(tc.tile_pool(name="ep", bufs=4))
    nrm_p = att_est.enter_context(tc.tile_pool(name="nrm", bufs=2))
    sq_p = att_est.enter_context(tc.tile_pool(name="sqp", bufs=2))
    bcd_p = att_est.enter_context(tc.tile_pool(name="bcd", bufs=2, space="DRAM"))

    st_est = ExitStack()
    st_ps = st_est.enter_context(tc.tile_pool(name="st_ps", bufs=1, space="PSUM"))
    stats = st_ps.tile([1, 2, TOK], F32, tag="stats")

    ps_est = ExitStack()
    s_ps = ps_est.enter_context(tc.tile_pool(name="s_ps", bufs=2, space="PSUM"))
    att_ps = ps_est.enter_context(tc.tile_pool(name="att_ps", bufs=1, space="PSUM"))

    def emit_stats(jprev, sq_tile):
        x1v = x1[:, jprev, :]
        nc.tensor.matmul(stats[:, 0, :], ones_col, x1v,
                         start=(jprev == 0), stop=(jprev == NPAIR - 1))
        nc.tensor.matmul(stats[:, 1, :], ones_col, sq_tile,
                         start=(jprev == 0), stop=(jprev == NPAIR - 1))

    pend_stats = None            # (jprev, sq_tile)
    for j in range(NPAIR):
        # ---- stage K/Q (gpsimd queue), V (sync queue) ----
        qt = kq_p.tile([128, TOK], BF16, tag="qt")
        nc.gpsimd.dma_start(out=qt, in_=qp[j])
        kg = kq_p.tile([128, KC, 128], BF16, tag="kg")
        nc.gpsimd.dma_start(out=kg[:, 0:KC // 2, :], in_=kp[j, :, 0:KC // 2])
        nc.gpsimd.dma_start(out=kg[:, KC // 2:, :], in_=kp[j, :, KC // 2:])
        va = v_p.tile([128, KC, HD + 1], F16, tag="va")
        nc.sync.dma_start(out=va, in_=vp[j, 0])
        vb = v_p.tile([128, KC, HD + 1], F16, tag="vb")
        nc.sync.dma_start(out=vb, in_=vp[j, 1])
        # big streaming loads ride the otherwise-idle sync queue, spread
        # across pairs so they never delay the next pair's V tiles; the
        # x slab for feature block j arrives just before its residual add
        nc.sync.dma_start(out=xta[:, j, :], in_=xt[:, j, :])
        if 1 <= j <= 4:
            sl = j - 1
            nc.sync.dma_start(
                out=w1a[:, ts(sl, FC // 4), :],
                in_=w1p[ts(sl, FC // 4)].rearrange("f p q -> p f q"))

        att_a = att_ps.tile([HD + 1, TOK], F32, tag="att_a")
        att_b = att_ps.tile([HD + 1, TOK], F32, tag="att_b")

        def exp_av(s, c):
            e = e_p.tile([128, 2, TOK], F16, tag="e")
            nc.scalar.activation(e, s, AF.Exp)
            nc.tensor.matmul(att_a, va[:, c, :], e[:, 0, :],
                             start=(c == 0), stop=(c == KC - 1))
            nc.tensor.matmul(att_b, vb[:, c, :], e[:, 1, :],
                             start=(c == 0), stop=(c == KC - 1))

        pend = None
        for c in range(KC):
            s = s_ps.tile([128, 2, TOK], F32, tag="s")
            nc.tensor.matmul(s[:, 0, :], kg[0:64, c, :], qt[0:64, :],
                             tile_position=(0, 0))
            nc.tensor.matmul(s[:, 1, :], kg[64:128, c, :], qt[64:128, :],
                             tile_position=(64, 0))
            if pend is not None:
                exp_av(*pend)
            pend = (s, c)
            # deferred LN stats for the previous pair, tucked into the
            # second chunk so the PE queue at the pair boundary stays busy
            if c == 2 and pend_stats is not None:
                emit_stats(*pend_stats)
                pend_stats = None
        exp_av(*pend)

        # ---- drain att PSUM to SBUF, reciprocal on DVE, broadcast ----
        cpa = nrm_p.tile([HD + 1, TOK], F32, tag="cpa")
        cpb = nrm_p.tile([HD + 1, TOK], F32, tag="cpb")
        nc.vector.tensor_copy(out=cpa, in_=att_a)
        nc.vector.tensor_copy(out=cpb, in_=att_b)
        if j == NPAIR - 1:
            break                # latency-optimized tail below
        # spread the 1024 denominators over all 128 partitions via a DRAM
        # round-trip so the DVE's iterative divide runs 8 elems/lane
        # (~0.25us) instead of 512 on one lane (~3.3us each).  The DMA-hop
        # latencies hide under the next pair's compute.
        bcd2 = bcd_p.tile([2, TOK], F32, tag="bcd2")
        bcd = bcd_p.tile([2, TOK], F32, tag="bcd")
        nc.gpsimd.dma_start(out=bcd[0:1, :], in_=cpa[HD:HD + 1, :])
        nc.gpsimd.dma_start(out=bcd[1:2, :], in_=cpb[HD:HD + 1, :])
        den128 = nrm_p.tile([128, 8], F32, tag="den128")
        nc.gpsimd.dma_start(
            out=den128, in_=bcd[:].rearrange("a (b f) -> (a b) f", f=8))
        rec128 = nrm_p.tile([128, 8], F32, tag="rec128")
        nc.vector.reciprocal(out=rec128, in_=den128)
        nc.gpsimd.dma_start(
            out=bcd2[:].rearrange("a (b f) -> (a b) f", f=8), in_=rec128)
        bca = nrm_p.tile([HD, TOK], F32, tag="bca")
        bcb = nrm_p.tile([HD, TOK], F32, tag="bcb")
        nc.gpsimd.dma_start(out=bca, in_=bcd2[0:1, :].to_broadcast((HD, TOK)))
        nc.gpsimd.dma_start(out=bcb, in_=bcd2[1:2, :].to_broadcast((HD, TOK)))

        # ---- normalize + residual into x1 (feature block j) ----
        nc.vector.tensor_mul(x1[0:HD, j, :], cpa[0:HD, :], bca)
        nc.vector.tensor_mul(x1[HD:128, j, :], cpb[0:HD, :], bcb)
        nc.vector.tensor_add(x1[0:HD, j, :], x1[0:HD, j, :], xta[0:HD, j, :])
        nc.vector.tensor_add(x1[HD:128, j, :], x1[HD:128, j, :],
                             xta[HD:128, j, :])

        # squares for the LN variance (DVE, off the ACT critical path)
        sq = sq_p.tile([128, TOK], F32R, tag="sq")
        nc.vector.tensor_mul(sq, x1[:, j, :], x1[:, j, :])
        pend_stats = (j, sq)

    # ======= latency-optimized last-pair tail =======
    # ACT is idle once the exp stream ends, so the last reciprocals run as
    # ln/exp there; every broadcast is a K=1 ones-row matmul into PSUM —
    # no DRAM round-trips on this serial chain.
    JL = NPAIR - 1
    rab = nrm_p.tile([1, 2, TOK], F32R, tag="rab")
    nc.scalar.activation(rab[:, 0, :], att_a[HD:HD + 1, :], AF.Ln)
    nc.scalar.activation(rab[:, 1, :], att_b[HD:HD + 1, :], AF.Ln)
    nc.scalar.activation(rab, rab, AF.Exp, scale=-1.0)
    ps_est.close()
    p7_est = ExitStack()
    p7_ps = p7_est.enter_context(tc.tile_pool(name="p7ps", bufs=1, space="PSUM"))
    bca7 = p7_ps.tile([HD, TOK], F32, tag="bca7")
    bcb7 = p7_ps.tile([HD, TOK], F32, tag="bcb7")
    nc.tensor.matmul(bca7, ones_row[:, 0:HD], rab[:, 0, :])
    nc.tensor.matmul(bcb7, ones_row[:, 0:HD], rab[:, 1, :])
    nc.vector.tensor_mul(x1[0:HD, JL, :], cpa[0:HD, :], bca7)
    nc.vector.tensor_mul(x1[HD:128, JL, :], cpb[0:HD, :], bcb7)
    nc.vector.tensor_add(x1[0:HD, JL, :], x1[0:HD, JL, :], xta[0:HD, JL, :])
    nc.vector.tensor_add(x1[HD:128, JL, :], x1[HD:128, JL, :],
                         xta[HD:128, JL, :])
    sq = sq_p.tile([128, TOK], F32R, tag="sq")
    nc.vector.tensor_mul(sq, x1[:, JL, :], x1[:, JL, :])
    emit_stats(JL, sq)

    # ---- layer-norm scalars ----
    mu = persist.tile([1, TOK], F32R, tag="mu")
    msq = persist.tile([1, TOK], F32, tag="msq")
    var = persist.tile([1, TOK], F32, tag="var")
    rstd = persist.tile([1, TOK], F32R, tag="rstd")
    nc.vector.tensor_scalar_mul(mu, stats[:, 0, :], 1.0 / D)
    nc.vector.tensor_scalar_mul(msq, stats[:, 1, :], 1.0 / D)
    nc.vector.tensor_mul(var, mu, mu)
    nc.vector.tensor_sub(var, msq, var)
    # rstd = exp(-0.5 * ln(var + eps)) -- stays within the ln/exp table set
    nc.scalar.activation(var, var, AF.Ln, bias=eps_t)
    nc.scalar.activation(rstd, var, AF.Exp, scale=-0.5)
    p7_est.close()
    st_est.close()
    att_est.close()

    # =================== FFN ===================
    ffn_est = ExitStack()
    tail_ps = ffn_est.enter_context(
        tc.tile_pool(name="tailps", bufs=1, space="PSUM"))
    mm_ps = ffn_est.enter_context(tc.tile_pool(name="mm_ps", bufs=6, space="PSUM"))
    ln_p = ffn_est.enter_context(tc.tile_pool(name="ln", bufs=2))
    w2_p = ffn_est.enter_context(tc.tile_pool(name="w2p", bufs=2))
    o_p = ffn_est.enter_context(tc.tile_pool(name="op", bufs=2))

    mu_b = tail_ps.tile([128, TOK], F32, tag="mu_b")
    rstd_b = tail_ps.tile([128, TOK], F32, tag="rstd_b")
    nc.tensor.matmul(mu_b, ones_row, mu)
    nc.tensor.matmul(rstd_b, ones_row, rstd)

    # LN apply: ht = (x1 - mu) * rstd   (ln_w/ln_b folded into w1/b1)
    for jj in range(DC):
        t = ln_p.tile([128, TOK], F32, tag="lnt")
        nc.vector.tensor_sub(t, x1[:, jj, :], mu_b)
        nc.vector.tensor_mul(ht[:, jj, :], t, rstd_b)

    # FFN1: gt[fc] = gelu(w1[:,fc]^T h + b1[fc])
    for f in range(FC):
        ps = mm_ps.tile([128, TOK], F32, tag="mm")
        for dc in range(DC):
            nc.tensor.matmul(ps, w1a[:, f, ts(dc, 128)], ht[:, dc, :],
                             start=(dc == 0), stop=(dc == DC - 1))
        nc.scalar.activation(gt[:, f, :], ps, AF.Gelu, bias=b1s[:, f:f + 1])

    # FFN2: y[dd] = w2[:,dd]^T g + b2[dd] + x1[dd]
    for dd in range(DC):
        w2t = w2_p.tile([128, FC * 128], BF16, tag="w2t")
        nc.sync.dma_start(out=w2t, in_=w2p[dd])
        ps = mm_ps.tile([128, TOK], F32, tag="mm")
        for fc in range(FC):
            nc.tensor.matmul(ps, w2t[:, ts(fc, 128)], gt[:, fc, :],
                             start=(fc == 0), stop=(fc == FC - 1))
        o = o_p.tile([128, TOK], F32, tag="o")
        nc.vector.tensor_scalar_add(o, ps, b2s[:, dd:dd + 1])
        nc.vector.tensor_add(o, o, x1[:, dd, :])
        nc.sync.dma_start(out=y[dd], in_=o)

    ffn_est.close()
    est.close()


_PROGRAMS = {}


def get_program(split_waits=True):
    if split_waits not in _PROGRAMS:
        _PROGRAMS[split_waits] = build_program(split_waits)
    return _PROGRAMS[split_waits]


def make_in_maps(x, image_q, image_k, image_v, ln_w, ln_b, w1, b1, w2, b2):
    f32 = np.float32
    bf16 = ml_dtypes.bfloat16
    x = np.asarray(x, f32)
    image_q = np.asarray(image_q, f32)
    image_k = np.asarray(image_k, f32)
    image_v = np.asarray(image_v, f32)
    w1 = np.asarray(w1, f32)
    b1 = np.asarray(b1, f32)
    w2 = np.asarray(w2, f32)
    b2 = np.asarray(b2, f32)
    ln_w = np.asarray(ln_w, f32)
    ln_b = np.asarray(ln_b, f32)

    # fold LN affine into w1/b1 (exact)
    w1f = ln_w[:, None] * w1
    b1f = b1 + ln_b @ w1
    w1p = np.ascontiguousarray(
        w1f.reshape(DC, 128, FC, 128).transpose(2, 1, 0, 3)
        .reshape(FC, 128, DC * 128).astype(bf16))
    w2p = np.ascontiguousarray(
        w2.reshape(FC, 128, DC, 128).transpose(2, 1, 0, 3)
        .reshape(DC, 128, FC * 128).astype(bf16))

    shared = {"w1p": w1p,
          "b1p": np.ascontiguousarray(b1f.reshape(FC, 128).T),
          "w2p": w2p,
          "b2p": np.ascontiguousarray(b2.reshape(DC, 128).T)}

    in_maps = []
    for core in range(NCORES):
        b, r = divmod(core, NCORES // B)
        rows = slice(TOK * r, TOK * (r + 1))
        # K dim-major, head-paired: [NPAIR, 128, KC, 128]
        kpa = np.ascontiguousarray(
            image_k[b].reshape(NPAIR, 2, KC, 128, HD)
            .transpose(0, 1, 4, 2, 3).reshape(NPAIR, 128, KC, 128)
            .astype(bf16))
        # Q dim-major, head-paired, scale folded: [NPAIR, 128, TOK]
        qpa = np.ascontiguousarray(
            (image_q[b, :, rows] * SCALE).reshape(NPAIR, 2, TOK, HD)
            .transpose(0, 1, 3, 2).reshape(NPAIR, 128, TOK)
            .astype(bf16))
        # V token-major per chunk with ones column: [NPAIR, 2, 128, KC, 65]
        vv = image_v[b].reshape(NPAIR, 2, KC, 128, HD).transpose(0, 1, 3, 2, 4)
        vpa = np.empty((NPAIR, 2, 128, KC, HD + 1), np.float16)
        vpa[..., :HD] = vv
        vpa[..., HD] = 1.0
        # x feature-major: [128, DC, TOK]
        xta = np.ascontiguousarray(
            x[b, rows].T.reshape(DC, 128, TOK).transpose(1, 0, 2))
        in_maps.append({
            "kp": kpa, "qp": qpa, "vp": vpa, "xt": xta, **shared,
        })
    return in_maps


def run_cores(in_maps, trace=False, **kw):
    nc = get_program()
    return run_bass_kernel_spmd(nc, in_maps, core_ids=list(range(NCORES)),
                                trace=trace, **kw)


def kernel(x, image_q, image_k, image_v, ln_w, ln_b, w1, b1, w2, b2):
    in_maps = make_in_maps(x, image_q, image_k, image_v, ln_w, ln_b,
                           w1, b1, w2, b2)
    res = run_cores(in_maps)
    out = np.empty((B, N, D), dtype=np.float32)
    for core in range(NCORES):
        b, r = divmod(core, NCORES // B)
        out[b, TOK * r:TOK * (r + 1)] = \
            np.asarray(res.results[core]["y"]).reshape(D, TOK).T
    return out
